# revision 23
# baseline (speedup 1.0000x reference)
"""Longformer self-attention Bass kernel for 8 trn2 NeuronCores.

Sharding: data-parallel over batch (2) x sequence-parallel over 4096 tokens
(4 chunks of 1024), with a 256-token halo recomputed locally per core.
Global-token attention (first 64 tokens attend to the full sequence) is
sequence-sharded with a host-side flash-style combine.

Device layout notes:
 - everything transposed: scores are computed as S^T[key, query] so softmax
   reduction lands in the matmul (ones-column in V gives sumexp for free)
   and no PE transposes are needed anywhere.
 - heads have dh=64, so even/odd head pairs occupy partition halves [0:64)
   and [64:128) and their QK matmuls row-tile the PE array concurrently.
 - per-key masks (out-of-range -1e9, global-key -10000) ride the Exp
   activation's per-partition bias; band-edge triangles are zeroed with
   affine_select.
Outputs are unnormalized [65 = 64 feat + sumexp, ...] blocks; the host
divides and reassembles.
"""

import math
import os

import numpy as np
import ml_dtypes

import concourse.bass as bass
import concourse.tile as tile
from concourse import mybir
from concourse.vector_clock import ScopedClock
from concourse.bass_utils import run_bass_kernel_spmd

F32 = mybir.dt.float32
BF16 = mybir.dt.bfloat16


# This container's walrus rejects instructions carrying more than one sync
# wait ("Too many sync wait commands" in setupSyncWait). Tile's scheduler
# freely attaches several. Post-process the serialized BIR: move excess
# waits onto single-wait NoOps inserted just before the instruction on the
# same engine (program order preserves the semantics).
import json as _json

_MAX_WAITS = 1


def _split_waits_json(bj_bytes):
    bj = _json.loads(bj_bytes)
    n_split = 0
    for f in bj["functions"]:
        for blk in f["blocks"]:
            out = []
            for ins in blk["instructions"]:
                si = ins.get("sync_info")
                ws = (si or {}).get("on_wait") or []
                if len(ws) > _MAX_WAITS:
                    for k, w in enumerate(ws[:-_MAX_WAITS]):
                        out.append({
                            "debug": ins.get("debug", 0),
                            "engine": ins["engine"],
                            "ins": [], "outs": [],
                            "name": f"{ins['name']}-xw{k}",
                            "opcode": "NoOp",
                            "sync_info": {"on_update": [], "on_wait": [w]},
                        })
                        n_split += 1
                    si["on_wait"] = ws[-_MAX_WAITS:]
                out.append(ins)
            blk["instructions"] = out
    return _json.dumps(bj).encode()


_orig_to_json = bass.Bass.to_json_bytes


def _patched_to_json(self, *a, **k):
    return _split_waits_json(_orig_to_json(self, *a, **k))


bass.Bass.to_json_bytes = _patched_to_json

B = 2
S = 4096
E = 768
H = 12
DH = 64
WIN = 256
G = 64
NCORES = 8
CH = 1024          # chunk length per core
HL = CH + 2 * WIN  # halo'd chunk length = 1536
FB = 6             # feature blocks of 128 (E = 768)
NQB = CH // 128    # 8 query blocks per core
NPB = HL // 128    # 12 halo position blocks
VW = H * 65        # interleaved v width (64 feats + ones col per head) = 780
SCALE = 1.0 / math.sqrt(DH)
NEG = -1e9

_nc_cache = None


def _build_nc(do_band=True, do_glob=True, nqb=NQB):
    nc = bass.Bass("TRN2", target_bir_lowering=False)

    hsT_d = nc.dram_tensor("hsT", [128, FB * HL], BF16, kind="ExternalInput")
    hsTg_d = nc.dram_tensor("hsTg", [128, FB * G], BF16, kind="ExternalInput")
    kb_d = nc.dram_tensor("kbias", [128, NQB * 5], F32, kind="ExternalInput")
    wq_d = nc.dram_tensor("wq", [128, 36 * 128], BF16, kind="ExternalInput")
    wk_d = nc.dram_tensor("wk", [128, 36 * 128], BF16, kind="ExternalInput")
    wkg_d = nc.dram_tensor("wkg", [128, 36 * 128], BF16, kind="ExternalInput")
    wqg_d = nc.dram_tensor("wqg", [128, 36 * 128], BF16, kind="ExternalInput")
    wv_d = nc.dram_tensor("wv", [128, FB * VW], BF16, kind="ExternalInput")
    wvg_d = nc.dram_tensor("wvg", [128, FB * VW], BF16, kind="ExternalInput")
    bvbc_d = nc.dram_tensor("bvbc", [128, VW], F32, kind="ExternalInput")
    bvgbc_d = nc.dram_tensor("bvgbc", [128, VW], F32, kind="ExternalInput")
    bq_d = nc.dram_tensor("bq", [128, FB], F32, kind="ExternalInput")
    bk_d = nc.dram_tensor("bk", [128, FB], F32, kind="ExternalInput")
    bkg_d = nc.dram_tensor("bkg", [128, FB], F32, kind="ExternalInput")
    bqg_d = nc.dram_tensor("bqg", [128, FB], F32, kind="ExternalInput")
    outa_d = nc.dram_tensor("outa", [NQB, 65, H * 128], F32, kind="ExternalOutput")
    og_d = nc.dram_tensor("og", [65, H * 64], F32, kind="ExternalOutput")

    # HW constraints discovered empirically on this axon stack:
    #  - all matmul operands must share partition base 0 (mixing base-0 and
    #    base-64 operands faults the device), so K-side tensors are stored
    #    per-head zero-padded to K=128 and the packed q supplies both head
    #    halves (the zero half kills sibling crosstalk);
    #  - SBUF tiles must be allocated with 128 partitions (sub-128 breaks DMA).
    with tile.TileContext(nc) as tc:
        with tc.tile_pool(name="sb", bufs=1) as sb:
            kbt = sb.tile([128, NQB * 5], F32)
            bq = sb.tile([128, FB], F32)
            bk = sb.tile([128, FB], F32)
            bkg = sb.tile([128, FB], F32)
            bqg = sb.tile([128, FB], F32)
            nc.sync.dma_start(kbt[:, :], kb_d[:, :])
            nc.sync.dma_start(bq[:, :], bq_d[:, :])
            nc.sync.dma_start(bk[:, :], bk_d[:, :])
            nc.sync.dma_start(bkg[:, :], bkg_d[:, :])
            nc.sync.dma_start(bqg[:, :], bqg_d[:, :])

            # persistent projection outputs
            qT = sb.tile([128, FB * CH], BF16)       # packed [feat, qpos]
            qgT = sb.tile([128, FB * G], BF16)       # packed global queries
            kTz = sb.tile([128, H * HL], BF16)       # per-head, half zeroed
            kgTz = sb.tile([128, H * CH], BF16)      # per-head, half zeroed
            kglTz = sb.tile([128, H * 128], BF16)    # 64 real + 64 fake keys/head
            vA = sb.tile([128, NPB * VW], BF16)      # v natural + ones cols
            vgA = sb.tile([128, NQB * VW], BF16)
            vglA = sb.tile([128, VW], BF16)          # rows 64: stay zero
            gbias = sb.tile([128, 1], F32)           # 0 / -1e9 fake-key bias

            nc.gpsimd.memset(kTz[:, :], 0.0)
            nc.gpsimd.memset(kgTz[:, :], 0.0)
            nc.vector.memset(kglTz[:, :], 0.0)
            nc.vector.memset(vglA[:, :], 0.0)
            nc.vector.memset(gbias[0:64, :], 0.0)
            nc.vector.memset(gbias[64:128, :], NEG)

            with tc.tile_pool(name="wp", bufs=1) as wp:
                hsT = wp.tile([128, FB * HL], BF16)
                hsTg = wp.tile([128, FB * G], BF16)
                wq = wp.tile([128, 36 * 128], BF16)
                wk = wp.tile([128, 36 * 128], BF16)
                wkg = wp.tile([128, 36 * 128], BF16)
                wqg = wp.tile([128, 36 * 128], BF16)
                wv = wp.tile([128, FB * VW], BF16)
                wvg = wp.tile([128, FB * VW], BF16)
                bvbc = wp.tile([128, VW], F32)
                bvgbc = wp.tile([128, VW], F32)
                half = FB * HL // 2
                nc.sync.dma_start(hsT[:, :half], hsT_d[:, :half])
                nc.sync.dma_start(hsT[:, half:], hsT_d[:, half:])
                nc.sync.dma_start(hsTg[:, :], hsTg_d[:, :])
                nc.sync.dma_start(wq[:, :], wq_d[:, :])
                nc.sync.dma_start(wk[:, :], wk_d[:, :])
                nc.sync.dma_start(wkg[:, :], wkg_d[:, :])
                nc.sync.dma_start(wqg[:, :], wqg_d[:, :])
                nc.sync.dma_start(wv[:, :], wv_d[:, :])
                nc.sync.dma_start(wvg[:, :], wvg_d[:, :])
                nc.sync.dma_start(bvbc[:, :], bvbc_d[:, :])
                nc.sync.dma_start(bvgbc[:, :], bvgbc_d[:, :])

                # ---------------- phase A: projections ----------------
                with tc.tile_pool(name="pp", bufs=3, space="PSUM") as pp, \
                     tc.tile_pool(name="ppv", bufs=2, space="PSUM") as ppv:

                    def proj_T(wl, bt, src, src_len, src_off, n_pos, out):
                        # packed: out[:, ob*n_pos + x] = (W.T @ hs)[ob*128+p, x] + b
                        for ob in range(FB):
                            for p0 in range(0, n_pos, 512):
                                pl = min(512, n_pos - p0)
                                ps = pp.tile([128, 512], F32, tag="pp")
                                for fb in range(FB):
                                    nc.tensor.matmul(
                                        ps[:, :pl],
                                        wl[:, (fb * FB + ob) * 128:
                                           (fb * FB + ob) * 128 + 128],
                                        src[:, fb * src_len + src_off + p0:
                                            fb * src_len + src_off + p0 + pl],
                                        start=(fb == 0), stop=(fb == FB - 1),
                                    )
                                nc.vector.tensor_scalar_add(
                                    out[:, ob * n_pos + p0:ob * n_pos + p0 + pl],
                                    ps[:, :pl], bt[:, ob:ob + 1])

                    def proj_Tz(wl, bt, src, src_len, src_off, n_pos, out, ostride):
                        # per-head zero-padded: head 2j from psum rows 0:64,
                        # head 2j+1 from rows 64:128; out col base h*ostride
                        for ob in range(FB):
                            for p0 in range(0, n_pos, 512):
                                pl = min(512, n_pos - p0)
                                ps = pp.tile([128, 512], F32, tag="pp")
                                for fb in range(FB):
                                    nc.tensor.matmul(
                                        ps[:, :pl],
                                        wl[:, (fb * FB + ob) * 128:
                                           (fb * FB + ob) * 128 + 128],
                                        src[:, fb * src_len + src_off + p0:
                                            fb * src_len + src_off + p0 + pl],
                                        start=(fb == 0), stop=(fb == FB - 1),
                                    )
                                h0, h1 = 2 * ob, 2 * ob + 1
                                nc.vector.tensor_scalar_add(
                                    out[0:64, h0 * ostride + p0:h0 * ostride + p0 + pl],
                                    ps[0:64, :pl], bt[0:64, ob:ob + 1])
                                nc.vector.tensor_scalar_add(
                                    out[64:128, h1 * ostride + p0:h1 * ostride + p0 + pl],
                                    ps[64:128, :pl], bt[64:128, ob:ob + 1])

                    def proj_N(wr, bbc, src, src_len, src_off, pos_blocks, m, out):
                        for pb in range(pos_blocks):
                            ps = ppv.tile([128, VW], F32, tag="ppv")
                            for c0, cl in ((0, 512), (512, VW - 512)):
                                for fb in range(FB):
                                    nc.tensor.matmul(
                                        ps[:m, c0:c0 + cl],
                                        src[:, fb * src_len + src_off + pb * 128:
                                            fb * src_len + src_off + pb * 128 + m],
                                        wr[:, fb * VW + c0:fb * VW + c0 + cl],
                                        start=(fb == 0), stop=(fb == FB - 1),
                                    )
                            nc.vector.tensor_add(
                                out[:m, pb * VW:pb * VW + VW] if pos_blocks > 1
                                else out[:m, 0:VW],
                                ps[:m, :], bbc[:m, :])

                    proj_Tz(wk, bk, hsT, HL, 0, HL, kTz, HL)
                    proj_T(wq, bq, hsT, HL, WIN, CH, qT)
                    proj_N(wv, bvbc, hsT, HL, 0, NPB, 128, vA)
                    proj_Tz(wk, bk, hsTg, G, 0, G, kglTz, 128)
                    proj_N(wv, bvbc, hsTg, G, 0, 1, 64, vglA)
                    proj_Tz(wkg, bkg, hsT, HL, WIN, CH, kgTz, CH)
                    proj_N(wvg, bvgbc, hsT, HL, WIN, NQB, 128, vgA)
                    proj_T(wqg, bqg, hsTg, G, 0, G, qgT)
            # wp closes: weights + hsT freed before attention pools open

            # ------------- phases B/C: attention -------------
            with tc.tile_pool(name="sm", bufs=2, space="PSUM") as smp, \
                 tc.tile_pool(name="pvp", bufs=2, space="PSUM") as pvp, \
                 tc.tile_pool(name="ptp", bufs=2) as ptp, \
                 tc.tile_pool(name="outp", bufs=3) as outp:

                for qb in range(nqb if do_band else 0):
                    for hg in range(2):
                        pT = ptp.tile([128, 6, 896], BF16, tag="pt")
                        pv = pvp.tile([65, 768], F32, tag="pv")
                        for rnd in range(6):
                            sm = smp.tile([128, 768], F32, tag="sm")
                            sm3 = sm.rearrange("p (g f) -> p g f", g=6)
                            if rnd < 5:
                                for hi in range(6):
                                    h = hg * 6 + hi
                                    nc.tensor.matmul(
                                        sm[:, hi * 128:hi * 128 + 128],
                                        kTz[:, h * HL + (qb + rnd) * 128:
                                            h * HL + (qb + rnd) * 128 + 128],
                                        qT[:, (h // 2) * CH + qb * 128:
                                           (h // 2) * CH + qb * 128 + 128],
                                        start=True, stop=True,
                                    )
                                nc.scalar.activation(
                                    pT[:, :, rnd * 128:rnd * 128 + 128],
                                    sm3[:, :, :],
                                    mybir.ActivationFunctionType.Exp,
                                    bias=kbt[:, qb * 5 + rnd:qb * 5 + rnd + 1],
                                )
                                if rnd == 0:
                                    # keep key - query >= 0 (partition - col)
                                    nc.gpsimd.affine_select(
                                        out=pT[:, :, 0:128], in_=pT[:, :, 0:128],
                                        pattern=[[0, 6], [-1, 128]],
                                        channel_multiplier=1, base=0,
                                        compare_op=mybir.AluOpType.is_ge, fill=0.0)
                                if rnd == 4:
                                    nc.gpsimd.affine_select(
                                        out=pT[:, :, 512:640], in_=pT[:, :, 512:640],
                                        pattern=[[0, 6], [1, 128]],
                                        channel_multiplier=-1, base=0,
                                        compare_op=mybir.AluOpType.is_ge, fill=0.0)
                            else:
                                for hi in range(6):
                                    h = hg * 6 + hi
                                    nc.tensor.matmul(
                                        sm[:, hi * 128:hi * 128 + 128],
                                        kglTz[:, h * 128:h * 128 + 128],
                                        qT[:, (h // 2) * CH + qb * 128:
                                           (h // 2) * CH + qb * 128 + 128],
                                        start=True, stop=True,
                                    )
                                nc.scalar.activation(
                                    pT[:, :, 768:896], sm3[:, :, :],
                                    mybir.ActivationFunctionType.Exp,
                                    bias=gbias[:, 0:1])
                            for hi in range(6):
                                h = hg * 6 + hi
                                if rnd < 5:
                                    lhsv = vA[:, (qb + rnd) * VW + h * 65:
                                              (qb + rnd) * VW + h * 65 + 65]
                                    rhsp = pT[:, hi, rnd * 128:rnd * 128 + 128]
                                else:
                                    lhsv = vglA[:, h * 65:h * 65 + 65]
                                    rhsp = pT[:, hi, 768:896]
                                # pv banks (2KB): start/stop only on the first/
                                # last matmul touching each bank
                                nc.tensor.matmul(
                                    pv[:, hi * 128:hi * 128 + 128], lhsv, rhsp,
                                    start=(rnd == 0 and hi in (0, 4)),
                                    stop=(rnd == 5 and hi in (3, 5)),
                                )
                        ot128 = outp.tile([128, 768], F32, tag="ot")
                        nc.vector.tensor_copy(ot128[0:65, :], pv[:, :])
                        nc.sync.dma_start(
                            outa_d[qb, :, hg * 768:(hg + 1) * 768], ot128[0:65, :])

                # ------------- phase C: global-token full attention ----------
                gv = pvp.tile([65, 768], F32, tag="pv")
                for kb in range(NQB if do_glob else 0):
                    gs = smp.tile([128, 768], F32, tag="sm")
                    for h in range(H):
                        nc.tensor.matmul(
                            gs[:, h * 64:h * 64 + 64],
                            kgTz[:, h * CH + kb * 128:h * CH + kb * 128 + 128],
                            qgT[:, (h // 2) * G:(h // 2) * G + 64],
                            start=True, stop=True,
                        )
                    pTg = ptp.tile([128, 768], BF16, tag="ptg")
                    nc.scalar.activation(pTg[:, :], gs[:, :],
                                         mybir.ActivationFunctionType.Exp)
                    for h in range(H):
                        # gv banks: cols 0:512 (h 0..7) and 512:768 (h 8..11)
                        nc.tensor.matmul(
                            gv[:, h * 64:h * 64 + 64],
                            vgA[:, kb * VW + h * 65:kb * VW + h * 65 + 65],
                            pTg[:, h * 64:h * 64 + 64],
                            start=(kb == 0 and h in (0, 8)),
                            stop=(kb == NQB - 1 and h in (7, 11)),
                        )
                if do_glob:
                    ogt128 = outp.tile([128, 768], F32, tag="ot")
                    nc.vector.tensor_copy(ogt128[0:65, :], gv[:, :])
                    nc.sync.dma_start(og_d[:, :], ogt128[0:65, :])

    return nc


def _tile_lhsT(w):
    # [768, 768] -> [128, 36*128] with col block (fb*6+ob)*128
    return np.ascontiguousarray(
        w.reshape(FB, 128, FB, 128).transpose(1, 0, 2, 3).reshape(128, 36 * 128)
    ).astype(ml_dtypes.bfloat16)


def _tile_rhs_v(w):
    # [768, 768] -> head-interleaved [768, 780] (+zero ones-cols) -> [128, 6*780]
    wi = np.zeros((E, VW), np.float32)
    for h in range(H):
        wi[:, h * 65:h * 65 + 64] = w[:, h * 64:(h + 1) * 64]
    return np.ascontiguousarray(
        wi.reshape(FB, 128, VW).transpose(1, 0, 2).reshape(128, FB * VW)
    ).astype(ml_dtypes.bfloat16)


def _bias_v_bcast(b):
    bi = np.zeros(VW, np.float32)
    for h in range(H):
        bi[h * 65:h * 65 + 64] = b[h * 64:(h + 1) * 64]
        bi[h * 65 + 64] = 1.0
    return np.ascontiguousarray(np.broadcast_to(bi, (128, VW)))


def _bias_col(b):
    return np.ascontiguousarray(b.reshape(FB, 128).T)


def _tile_hsT(x, n):
    # x [n, 768] -> [128, 6*n]
    return np.ascontiguousarray(
        x.T.reshape(FB, 128, n).transpose(1, 0, 2).reshape(128, FB * n)
    ).astype(ml_dtypes.bfloat16)


def kernel(**inputs):
    global _nc_cache
    hs = np.asarray(inputs["hidden_states"], np.float32)
    am = np.asarray(inputs["attention_mask"], np.float32).reshape(B, S)
    fmask = np.where(am != 0, -10000.0, 0.0).astype(np.float32)

    shared = {
        "wq": _tile_lhsT(np.asarray(inputs["Wq"], np.float32) * SCALE),
        "wk": _tile_lhsT(np.asarray(inputs["Wk"], np.float32)),
        "wkg": _tile_lhsT(np.asarray(inputs["Wkg"], np.float32)),
        "wqg": _tile_lhsT(np.asarray(inputs["Wqg"], np.float32) * SCALE),
        "wv": _tile_rhs_v(np.asarray(inputs["Wv"], np.float32)),
        "wvg": _tile_rhs_v(np.asarray(inputs["Wvg"], np.float32)),
        "bvbc": _bias_v_bcast(np.asarray(inputs["bv"], np.float32)),
        "bvgbc": _bias_v_bcast(np.asarray(inputs["bvg"], np.float32)),
        "bq": _bias_col(np.asarray(inputs["bq"], np.float32) * SCALE),
        "bk": _bias_col(np.asarray(inputs["bk"], np.float32)),
        "bkg": _bias_col(np.asarray(inputs["bkg"], np.float32)),
        "bqg": _bias_col(np.asarray(inputs["bqg"], np.float32) * SCALE),
    }

    in_maps = []
    for c in range(NCORES):
        b, j = divmod(c, NCORES // B)
        cs = j * CH
        halo = np.zeros((HL, E), np.float32)
        lo, hi = cs - WIN, cs + CH + WIN
        slo, shi = max(lo, 0), min(hi, S)
        halo[slo - lo:shi - lo] = hs[b, slo:shi]

        kbias = np.zeros((NQB, 5, 128), np.float32)
        for qb in range(NQB):
            for m in range(5):
                keys = cs + (qb + m) * 128 - WIN + np.arange(128)
                valid = (keys >= 0) & (keys < S)
                kc = np.clip(keys, 0, S - 1)
                kbias[qb, m] = np.where(valid, fmask[b, kc], NEG)

        in_maps.append({
            **shared,
            "hsT": _tile_hsT(halo, HL),
            "hsTg": _tile_hsT(hs[b, :G], G),
            "kbias": np.ascontiguousarray(kbias.transpose(2, 0, 1).reshape(128, NQB * 5)),
        })

    if _nc_cache is None:
        _nc_cache = _build_nc()
    res = run_bass_kernel_spmd(_nc_cache, in_maps, core_ids=list(range(NCORES)))
    kernel._last_results = res

    out = np.empty((B, S, E), np.float32)
    og_acc = [np.zeros((65, H, 64), np.float64) for _ in range(B)]
    for c in range(NCORES):
        b, j = divmod(c, NCORES // B)
        r = res.results[c]
        oa = r["outa"].reshape(NQB, 65, H, 128)
        o = oa[:, :64]                          # [qb, d, h, i]
        l = oa[:, 64]                           # [qb, h, i]
        blk = o.transpose(0, 3, 2, 1) / l.transpose(0, 2, 1)[..., None]
        out[b, j * CH:(j + 1) * CH] = blk.reshape(CH, E)
        og_acc[b] += r["og"].reshape(65, H, 64)
    for b in range(B):
        o = og_acc[b][:64]                      # [d, h, q]
        l = og_acc[b][64]                       # [h, q]
        out[b, :G] = (o / l[None]).transpose(2, 1, 0).reshape(G, E)
    return out
